# revision 1
# baseline (speedup 1.0000x reference)
"""Trainium2 Bass kernel for the 4-layer LIF spiking network (EventDrivenSparseNetwork).

Strategy:
  - Data-parallel over batch: B=32 sharded 4-per-core across 8 NeuronCores,
    weights replicated. No cross-core communication.
  - Per core and layer: dense GEMM cur = acts @ (W*mask)^T on TensorE, then
    the LIF scan on VectorE: v = alpha*v + cur_t; s = (v >= 1); v = (v < 1)*v.
  - Numerical fidelity is critical (threshold crossings make the network
    chaotic; matmul noise >= ~1e-5 flips spikes and cascades):
      * layer 0 (gaussian input) uses true fp32 matmuls;
      * layers 1-3 (exact 0/1 spike inputs) can use a 3-term fp16 weight
        split w ~= h0 + 2^-11*h1 + 2^-22*h2 accumulated into 3 PSUM banks
        (mode "fp16x3"), each term a full-rate fp16 matmul -- the products
        are exact (spike is 0/1), so effective weight precision is ~33 bits,
        beyond fp32.
  - Activations live in SBUF as [128p, 16no, 400f] with n = no*128+p and
    f = t*4 + b so the scan's per-timestep slice is a cheap strided AP and a
    layer's spikes feed the next layer's GEMM with no reshuffling.
  - T is processed in 2 chunks so the scan of chunk c overlaps the GEMM of
    chunk c+1 / next layer's first chunk, keeping TensorE dense (weights are
    re-streamed per chunk; HBM has the headroom).
  - All transposes / masking / sharding happen host-side in numpy; the
    device sees contiguous DMA-friendly layouts.
"""

import os
import sys

sys.path.insert(0, "/opt/trn_rl_repo")

import numpy as np

B, T, N = 32, 100, 2048
NL = 4
NCORES = 8
BL = B // NCORES          # 4 samples per core
NO = N // 128             # 16 output-neuron chunks
KO = N // 128             # 16 contraction chunks
F = T * BL                # 400, f = t*BL + b
ALPHA = float(np.float32(np.exp(np.float32(-1.0 / 20.0))))
CHUNKS = 2
# Uneven T-chunks: fat first chunk amortizes per-matmul weight loads; the
# small tail chunk still lets the scan overlap the next chunk's GEMM.
TSPLIT = tuple(int(t) for t in os.environ.get("LIF_TSPLIT", "50,50").split(","))
# Layer-0 (fp32) prefers a fat first chunk (amortizes its 2-pass weight
# loads); the fp16 layers prefer even chunks (measured).
TSPLIT0 = tuple(int(t)
                for t in os.environ.get("LIF_TSPLIT0", "64,36").split(","))
MODE = os.environ.get("LIF_MODE", "fp32")   # "fp32" | "fp16x3"
# WPASS=2: weights re-streamed per chunk (GEMM of chunk c fully precedes c+1,
#          scan overlaps the next chunk's GEMM). WPASS=1: one weight DMA per
#          mo with both chunks' GEMMs back-to-back (halves weight DMA, but
#          the scan serializes after the layer's GEMM).
WPASS = int(os.environ.get("LIF_WPASS", "2"))
S1, S2 = float(2.0 ** -11), float(2.0 ** -22)


def build(reps: int = 1, tsplit=TSPLIT, tsplit0=TSPLIT0,
          internal_weights: bool = False, mode: str = MODE):
    """Build (and bacc-compile) the SPMD kernel. Returns the Bass object."""
    import contextlib
    from concourse import mybir, bacc
    import concourse.tile as tile

    f16 = mode == "fp16x3"
    assert sum(tsplit) == T and sum(tsplit0) == T
    if not f16:
        tsplit = tsplit0
    layer_tsplits = [tsplit0 if (f16 and l == 0) else tsplit
                     for l in range(NL)]
    fmax = max(t * BL for ts_ in layer_tsplits for t in ts_)

    nc = bacc.Bacc("TRN2", target_bir_lowering=False, debug=False,
                   num_devices=NCORES)
    wkind = {} if internal_weights else {"kind": "ExternalInput"}
    w_d = nc.dram_tensor("w", [NO, 128, KO, 128], mybir.dt.float32,
                         **wkind).ap()       # layer-0 fp32 weights
    if f16:
        wh_d = nc.dram_tensor("wh", [NL - 1, NO, 128, 3, KO, 128],
                              mybir.dt.float16, **wkind).ap()
    else:
        wh_d = nc.dram_tensor("wh", [NL - 1, NO, 128, KO, 128],
                              mybir.dt.float32, **wkind).ap()
    x_d = nc.dram_tensor("x", [128, KO, F], mybir.dt.float32,
                         kind="ExternalInput").ap()
    out_d = nc.dram_tensor("out", [128, NO, F], mybir.dt.float32,
                           kind="ExternalOutput").ap()

    adt = mybir.dt.float16 if f16 else mybir.dt.float32

    with tile.TileContext(nc) as tctx:
        with contextlib.ExitStack() as stack:
            actsp = stack.enter_context(tctx.tile_pool(name="acts", bufs=3))
            wp = stack.enter_context(tctx.tile_pool(name="wp", bufs=6))
            curp = stack.enter_context(tctx.tile_pool(name="curp", bufs=2))
            vp = stack.enter_context(tctx.tile_pool(name="vp", bufs=2))
            tp = stack.enter_context(tctx.tile_pool(name="tp", bufs=4))
            pp = stack.enter_context(tctx.tile_pool(name="pp", bufs=4,
                                                    space="PSUM"))

            def load_w(l, mo):
                if f16 and l > 0:
                    wt = wp.tile([128, 3, KO, 128], mybir.dt.float16, tag="w")
                    nc.sync.dma_start(wt[:, :, :, :], wh_d[l - 1, mo])
                else:
                    wt = wp.tile([128, KO, 128], mybir.dt.float32, tag="w")
                    nc.sync.dma_start(wt[:, :, :],
                                      w_d[mo] if l == 0 else wh_d[l - 1, mo])
                return wt

            PBUFS = (3, 3, 2)

            def gemm_chunk_fp32(l, mo, cur, cur_in, f0, fc, wt):
                pt = pp.tile([128, fc], mybir.dt.float32, tag=f"p{mo % 3}",
                             name=f"ptf_{l}_{mo}_{f0}", bufs=PBUFS[mo % 3])
                for ko in range(KO):
                    nc.tensor.matmul(pt[:, :], wt[:, ko, :],
                                     cur_in[:, ko, f0:f0 + fc],
                                     start=(ko == 0), stop=(ko == KO - 1))
                nc.scalar.copy(cur[:, mo, :fc], pt[:, :])

            def gemm_chunk_f16(l, mo, cur, cur_in, f0, fc, wt):
                pts = [pp.tile([128, fc], mybir.dt.float32, tag=f"p{i}",
                               name=f"pt{i}_{l}_{mo}_{f0}", bufs=PBUFS[i])
                       for i in range(3)]
                for ko in range(KO):
                    for i in range(3):
                        nc.tensor.matmul(pts[i][:, :], wt[:, i, ko, :],
                                         cur_in[:, ko, f0:f0 + fc],
                                         start=(ko == 0), stop=(ko == KO - 1))
                t2 = tp.tile([128, fc], mybir.dt.float32, tag="t2")
                t12 = tp.tile([128, fc], mybir.dt.float32, tag="t12")
                nc.scalar.mul(t2[:, :], pts[2][:, :], S2)
                nc.vector.scalar_tensor_tensor(
                    t12[:, :], pts[1][:, :], S1, t2[:, :],
                    op0=mybir.AluOpType.mult, op1=mybir.AluOpType.add)
                nc.vector.scalar_tensor_tensor(
                    cur[:, mo, :fc], t12[:, :], 1.0, pts[0][:, :],
                    op0=mybir.AluOpType.mult, op1=mybir.AluOpType.add)

            def body(_iv=None):
                acts = actsp.tile([128, KO, F], mybir.dt.float32, tag="acts")
                for kg in range(4):
                    nc.sync.dma_start(acts[:, kg * 4:(kg + 1) * 4, :],
                                      x_d[:, kg * 4:(kg + 1) * 4, :])
                cur_in = acts

                def gemm(l, mo, cur, f0, fc, wt):
                    if f16 and l > 0:
                        gemm_chunk_f16(l, mo, cur, cur_in, f0, fc, wt)
                    else:
                        gemm_chunk_fp32(l, mo, cur, cur_in, f0, fc, wt)

                def scan(l, spk, vt, cur, f0, tcs):
                    for ts in range(tcs):
                        tl = slice(ts * BL, (ts + 1) * BL)
                        gl = slice(f0 + ts * BL, f0 + (ts + 1) * BL)
                        nc.vector.scalar_tensor_tensor(
                            vt[:, :, :], vt[:, :, :], ALPHA, cur[:, :, tl],
                            op0=mybir.AluOpType.mult, op1=mybir.AluOpType.add)
                        nc.vector.tensor_scalar(
                            spk[:, :, gl], vt[:, :, :], 1.0, None,
                            op0=mybir.AluOpType.is_ge)
                        nc.vector.scalar_tensor_tensor(
                            vt[:, :, :], vt[:, :, :], 1.0, vt[:, :, :],
                            op0=mybir.AluOpType.is_lt,
                            op1=mybir.AluOpType.mult)

                for l in range(NL):
                    fsplit = [t * BL for t in layer_tsplits[l]]
                    foffs = [sum(fsplit[:i]) for i in range(len(fsplit))]
                    tsp_l = layer_tsplits[l]
                    sdt = mybir.dt.float32 if (l == NL - 1 or not f16) else adt
                    spk = actsp.tile([128, NO, F], sdt, tag="acts")
                    vt = vp.tile([128, NO, BL], mybir.dt.float32, tag="v")
                    nc.vector.memset(vt[:, :, :], 0.0)
                    if WPASS == 1:
                        curs = [curp.tile([128, NO, fmax],
                                          mybir.dt.float32, tag=f"cur{c}",
                                          name=f"cur_{l}_{c}")
                                for c in range(len(fsplit))]
                        for mo in range(NO):
                            wt = load_w(l, mo)
                            for c, (f0, fc, tcs) in enumerate(
                                    zip(foffs, fsplit, tsp_l)):
                                gemm(l, mo, curs[c], f0, fc, wt)
                        for c, (f0, fc, tcs) in enumerate(
                                zip(foffs, fsplit, tsp_l)):
                            scan(l, spk, vt, curs[c], f0, tcs)
                            if l == NL - 1:
                                nc.sync.dma_start(out_d[:, :, f0:f0 + fc],
                                                  spk[:, :, f0:f0 + fc])
                    else:
                        for c, (f0, fc, tcs) in enumerate(
                                zip(foffs, fsplit, tsp_l)):
                            cur = curp.tile([128, NO, fmax],
                                            mybir.dt.float32, tag="cur",
                                            name=f"cur_{l}_{c}")
                            for mo in range(NO):
                                wt = load_w(l, mo)
                                gemm(l, mo, cur, f0, fc, wt)
                            scan(l, spk, vt, cur, f0, tcs)
                            if l == NL - 1:
                                nc.sync.dma_start(out_d[:, :, f0:f0 + fc],
                                                  spk[:, :, f0:f0 + fc])
                    cur_in = spk

            if reps == 1:
                body()
            else:
                with tctx.For_i(0, reps, 1) as iv:
                    body(iv)
    nc.compile()
    return nc


def _chunk(wm):
    """Wm [m, n] fp32 -> [mo, p, ko, mi] contiguous with lhsT layout."""
    wmT = np.ascontiguousarray(wm.T)
    return np.ascontiguousarray(
        wmT.reshape(KO, 128, NO, 128).transpose(2, 1, 0, 3))


def prep_weights(inputs, mode: str = MODE):
    """Returns dict of weight arrays for in_maps."""
    wms = []
    for l in range(NL):
        wm = (np.asarray(inputs[f"W{l}"], np.float32)
              * np.asarray(inputs[f"mask{l}"]).astype(np.float32))
        wms.append(wm)
    w0 = _chunk(wms[0])
    if mode == "fp16x3":
        wh = np.empty((NL - 1, NO, 128, 3, KO, 128), np.float16)
        for l in range(1, NL):
            wc = _chunk(wms[l]).astype(np.float32)
            h0 = wc.astype(np.float16)
            r1 = wc - h0.astype(np.float32)
            h1 = (r1 * np.float32(2.0 ** 11)).astype(np.float16)
            r2 = r1 - h1.astype(np.float32) * np.float32(2.0 ** -11)
            h2 = (r2 * np.float32(2.0 ** 22)).astype(np.float16)
            wh[l - 1, :, :, 0] = h0
            wh[l - 1, :, :, 1] = h1
            wh[l - 1, :, :, 2] = h2
        return {"w": w0, "wh": wh}
    wh = np.stack([_chunk(wms[l]) for l in range(1, NL)])
    return {"w": w0, "wh": wh}


def prep_x(x_core):
    """x_core [BL, T, N] -> [128, KO, F] with f = t*BL+b, n = no*128+p."""
    xt = x_core.transpose(2, 1, 0)                 # [n, t, b]
    xt = xt.reshape(KO, 128, T, BL).transpose(1, 0, 2, 3)  # [p, no, t, b]
    return np.ascontiguousarray(xt.reshape(128, KO, F), dtype=np.float32)


def unprep_out(o):
    """[128, NO, F] -> [BL, T, N]."""
    o = o.reshape(128, NO, T, BL).transpose(1, 0, 2, 3)    # [no, p, t, b]
    o = o.reshape(N, T, BL).transpose(2, 1, 0)             # [b, t, n]
    return np.ascontiguousarray(o)


_cached_nc = None


def kernel(**inputs) -> np.ndarray:
    global _cached_nc
    from concourse.bass_utils import run_bass_kernel_spmd

    if _cached_nc is None:
        _cached_nc = build(reps=1)
    nc = _cached_nc

    wmaps = prep_weights(inputs)
    x = np.asarray(inputs["x"], np.float32)
    in_maps = [dict(wmaps, x=prep_x(x[ci * BL:(ci + 1) * BL]))
               for ci in range(NCORES)]
    res = run_bass_kernel_spmd(nc, in_maps, core_ids=list(range(NCORES)))
    out = np.empty((B, T, N), np.float32)
    for ci in range(NCORES):
        out[ci * BL:(ci + 1) * BL] = unprep_out(res.results[ci]["out"])
    return out



# revision 3
# speedup vs baseline: 1.6525x; 1.6525x over previous
"""Trainium2 Bass kernel for the 4-layer LIF spiking network (EventDrivenSparseNetwork).

Strategy:
  - Data-parallel over batch: B=32 sharded 4-per-core across 8 NeuronCores,
    weights replicated. No cross-core communication.
  - All GEMMs run at full PE rate (1 cycle/row) via fp16 term-splitting;
    fp32 matmuls (4 cycles/row) are eliminated entirely:
      * layers 1-3: inputs are exact 0/1 spikes (stored fp16), weights are
        split w*2^12 ~= h0 + 2^-11*h1 (fp16 each) -> 2 full-rate matmuls
        into 2 PSUM banks; products are exact, so effective weight
        precision is ~22 bits (+ optional 3rd term -> 33 bits).
      * layer 0: gaussian input x is also split x ~= x0 + 2^-11*x1 (fp16),
        cur = h0x0 + 2^-11(h0x1 + h1x0) [+ 2^-22 h1x1] -> 3 (or 4)
        full-rate matmuls; dropped terms are O(2^-22), i.e. fp32-noise.
  - Weight terms are pre-scaled by 2^12 so h1 (and tiny h0) stay out of
    the fp16 subnormal range; the LIF threshold becomes 2^12 (power-of-2
    scaling is exact in fp32, so scan numerics are unchanged).
  - The LIF scan runs on VectorE: v = alpha*v + cur_t; s = (v >= TH);
    v = (v < TH)*v.  T is processed in 2 chunks so the scan of chunk c
    overlaps the GEMM of chunk c+1 / the next layer's first chunk.
  - All transposes / masking / splitting / sharding happen host-side in
    numpy; the device sees contiguous DMA-friendly layouts.
"""

import os
import sys

sys.path.insert(0, "/opt/trn_rl_repo")

import numpy as np

B, T, N = 32, 100, 2048
NL = 4
NCORES = 8
BL = B // NCORES          # 4 samples per core
NO = N // 128             # 16 output-neuron chunks
KO = N // 128             # 16 contraction chunks
F = T * BL                # 400, f = t*BL + b
ALPHA = float(np.float32(np.exp(np.float32(-1.0 / 20.0))))
TSPLIT = tuple(int(t) for t in os.environ.get("LIF_TSPLIT", "50,50").split(","))
NT = int(os.environ.get("LIF_NTERMS", "2"))      # weight fp16 terms (2 or 3)
L0MM = int(os.environ.get("LIF_L0MM", "3"))      # layer-0 matmuls (3 or 4)
WSCALE = float(2.0 ** 12)
THRESH = float(2.0 ** 12)
S1 = float(2.0 ** -11)


def build(reps: int = 1, tsplit=TSPLIT, nterms=NT, l0mm=L0MM):
    """Build (and bacc-compile) the SPMD kernel. Returns the Bass object."""
    import contextlib
    from concourse import mybir, bacc
    import concourse.tile as tile

    assert sum(tsplit) == T
    fsplit = [t * BL for t in tsplit]
    foffs = [sum(fsplit[:i]) for i in range(len(fsplit))]
    fmax = max(fsplit)
    f16 = mybir.dt.float16

    nc = bacc.Bacc("TRN2", target_bir_lowering=False, debug=False,
                   num_devices=NCORES)
    wh_d = nc.dram_tensor("wh", [NL, NO, 128, nterms, KO, 128], f16,
                          kind="ExternalInput").ap()
    x_d = nc.dram_tensor("x", [2, 128, KO, F], f16,
                         kind="ExternalInput").ap()
    out_d = nc.dram_tensor("out", [128, NO, F], mybir.dt.float32,
                           kind="ExternalOutput").ap()

    with tile.TileContext(nc) as tctx:
        with contextlib.ExitStack() as stack:
            actsp = stack.enter_context(tctx.tile_pool(name="acts", bufs=3))
            xp = stack.enter_context(tctx.tile_pool(name="xp", bufs=1))
            wp = stack.enter_context(tctx.tile_pool(name="wp", bufs=6))
            curp = stack.enter_context(tctx.tile_pool(name="curp", bufs=2))
            vp = stack.enter_context(tctx.tile_pool(name="vp", bufs=2))
            tp = stack.enter_context(tctx.tile_pool(name="tp", bufs=2))
            pp = stack.enter_context(tctx.tile_pool(name="pp", bufs=4,
                                                    space="PSUM"))

            PBUFS = (3, 3, 2)

            def body(_iv=None):
                xt = xp.tile([128, 2, KO, F], f16, tag="x01")
                for h in range(2):
                    nc.sync.dma_start(xt[:, h, :, :], x_d[h])

                def gemm(l, mo, cur, cur_in, f0, fc, wt):
                    np_ = 3 if (l == 0 and l0mm == 4) or \
                        (l > 0 and nterms == 3) else 2
                    pts = [pp.tile([128, fc], mybir.dt.float32, tag=f"p{i}",
                                   name=f"pt{i}_{l}_{mo}_{f0}", bufs=PBUFS[i])
                           for i in range(np_)]
                    if l == 0:
                        x0 = xt[:, 0]
                        x1 = xt[:, 1]
                        for ko in range(KO):
                            st, sp = ko == 0, ko == KO - 1
                            nc.tensor.matmul(pts[0][:, :], wt[:, 0, ko, :],
                                             x0[:, ko, f0:f0 + fc],
                                             start=st, stop=sp)
                            nc.tensor.matmul(pts[1][:, :], wt[:, 0, ko, :],
                                             x1[:, ko, f0:f0 + fc],
                                             start=st, stop=False)
                            nc.tensor.matmul(pts[1][:, :], wt[:, 1, ko, :],
                                             x0[:, ko, f0:f0 + fc],
                                             start=False, stop=sp)
                            if l0mm == 4:
                                nc.tensor.matmul(pts[2][:, :],
                                                 wt[:, 1, ko, :],
                                                 x1[:, ko, f0:f0 + fc],
                                                 start=st, stop=sp)
                    else:
                        for ko in range(KO):
                            st, sp = ko == 0, ko == KO - 1
                            for i in range(np_):
                                nc.tensor.matmul(pts[i][:, :],
                                                 wt[:, i, ko, :],
                                                 cur_in[:, ko, f0:f0 + fc],
                                                 start=st, stop=sp)
                    # ScalarE moves the scaled low term PSUM->SBUF, then one
                    # DVE op adds the top term (only one PSUM input allowed).
                    t2 = tp.tile([128, fc], mybir.dt.float32, tag="t2")
                    if np_ == 2:
                        nc.scalar.mul(t2[:, :], pts[1][:, :], S1)
                    else:
                        nc.scalar.mul(t2[:, :], pts[2][:, :], S1 * S1)
                        nc.vector.scalar_tensor_tensor(
                            t2[:, :], pts[1][:, :], S1, t2[:, :],
                            op0=mybir.AluOpType.mult,
                            op1=mybir.AluOpType.add)
                    nc.vector.scalar_tensor_tensor(
                        cur[:, mo, :fc], t2[:, :], 1.0, pts[0][:, :],
                        op0=mybir.AluOpType.mult,
                        op1=mybir.AluOpType.add)

                def scan(l, spk, vt, cur, f0, tcs):
                    for ts in range(tcs):
                        tl = slice(ts * BL, (ts + 1) * BL)
                        gl = slice(f0 + ts * BL, f0 + (ts + 1) * BL)
                        nc.vector.scalar_tensor_tensor(
                            vt[:, :, :], vt[:, :, :], ALPHA, cur[:, :, tl],
                            op0=mybir.AluOpType.mult, op1=mybir.AluOpType.add)
                        nc.vector.tensor_scalar(
                            spk[:, :, gl], vt[:, :, :], THRESH, None,
                            op0=mybir.AluOpType.is_ge)
                        nc.vector.scalar_tensor_tensor(
                            vt[:, :, :], vt[:, :, :], THRESH, vt[:, :, :],
                            op0=mybir.AluOpType.is_lt,
                            op1=mybir.AluOpType.mult)

                cur_in = None
                for l in range(NL):
                    sdt = mybir.dt.float32 if l == NL - 1 else f16
                    spk = actsp.tile([128, NO, F], sdt, tag="acts")
                    vt = vp.tile([128, NO, BL], mybir.dt.float32, tag="v")
                    nc.vector.memset(vt[:, :, :], 0.0)
                    for c, (f0, fc, tcs) in enumerate(
                            zip(foffs, fsplit, tsplit)):
                        cur = curp.tile([128, NO, fmax],
                                        mybir.dt.float32, tag="cur",
                                        name=f"cur_{l}_{c}")
                        for mo in range(NO):
                            wt = wp.tile([128, nterms, KO, 128], f16,
                                         tag="w")
                            nc.sync.dma_start(wt[:, :, :, :], wh_d[l, mo])
                            gemm(l, mo, cur, cur_in, f0, fc, wt)
                        scan(l, spk, vt, cur, f0, tcs)
                        if l == NL - 1:
                            nc.sync.dma_start(out_d[:, :, f0:f0 + fc],
                                              spk[:, :, f0:f0 + fc])
                    cur_in = spk

            if reps == 1:
                body()
            else:
                with tctx.For_i(0, reps, 1) as iv:
                    body(iv)
    nc.compile()
    return nc


def _chunk(wm):
    """Wm [m, n] fp32 -> [mo, p, ko, mi] contiguous with lhsT layout."""
    wmT = np.ascontiguousarray(wm.T)
    return np.ascontiguousarray(
        wmT.reshape(KO, 128, NO, 128).transpose(2, 1, 0, 3))


def prep_weights(inputs, nterms=NT):
    """Returns dict of weight arrays for in_maps: fp16 term-split, 2^12-scaled."""
    wh = np.empty((NL, NO, 128, nterms, KO, 128), np.float16)
    for l in range(NL):
        wm = (np.asarray(inputs[f"W{l}"], np.float32)
              * np.asarray(inputs[f"mask{l}"]).astype(np.float32))
        wc = _chunk(wm).astype(np.float32) * np.float32(WSCALE)
        h0 = wc.astype(np.float16)
        r1 = wc - h0.astype(np.float32)
        h1 = (r1 * np.float32(2.0 ** 11)).astype(np.float16)
        wh[l, :, :, 0] = h0
        wh[l, :, :, 1] = h1
        if nterms == 3:
            r2 = r1 - h1.astype(np.float32) * np.float32(2.0 ** -11)
            wh[l, :, :, 2] = (r2 * np.float32(2.0 ** 22)).astype(np.float16)
    return {"wh": wh}


def prep_x(x_core):
    """x_core [BL, T, N] -> [2, 128, KO, F] fp16 split, f = t*BL+b."""
    xt = x_core.transpose(2, 1, 0)                 # [n, t, b]
    xt = xt.reshape(KO, 128, T, BL).transpose(1, 0, 2, 3)  # [p, ko, t, b]
    xt = np.ascontiguousarray(xt.reshape(128, KO, F), dtype=np.float32)
    x0 = xt.astype(np.float16)
    x1 = ((xt - x0.astype(np.float32))
          * np.float32(2.0 ** 11)).astype(np.float16)
    return np.stack([x0, x1])


def unprep_out(o):
    """[128, NO, F] -> [BL, T, N]."""
    o = o.reshape(128, NO, T, BL).transpose(1, 0, 2, 3)    # [no, p, t, b]
    o = o.reshape(N, T, BL).transpose(2, 1, 0)             # [b, t, n]
    return np.ascontiguousarray(o)


_cached_nc = None


def kernel(**inputs) -> np.ndarray:
    global _cached_nc
    from concourse.bass_utils import run_bass_kernel_spmd

    if _cached_nc is None:
        _cached_nc = build(reps=1)
    nc = _cached_nc

    wmaps = prep_weights(inputs)
    x = np.asarray(inputs["x"], np.float32)
    in_maps = [dict(wmaps, x=prep_x(x[ci * BL:(ci + 1) * BL]))
               for ci in range(NCORES)]
    res = run_bass_kernel_spmd(nc, in_maps, core_ids=list(range(NCORES)))
    out = np.empty((B, T, N), np.float32)
    for ci in range(NCORES):
        out[ci * BL:(ci + 1) * BL] = unprep_out(res.results[ci]["out"])
    return out
